# revision 1
# baseline (speedup 1.0000x reference)
"""SSIM loss kernel for Trainium2, SPMD over 8 NeuronCores.

Inputs: img1, img2 [16,3,512,512] f32. Output: scalar mean SSIM (f32).
Sharding: batch dim 16 -> 2 per core; host sums per-core partial sums.

Math (per pixel, 11x11 Gaussian window, C1=1e-4, C2=9e-4):
  ma = conv(x)+conv(y), mb = conv(x)-conv(y)   (PSUM-accumulated matmuls)
  cD = 2*conv(x*y), cS = conv(x^2+y^2)         (cS PSUM-accumulated)
  A = ma^2/2, B = mb^2/2  ->  r2 = A-B = 2 mu1 mu2, msq = A+B = mu1^2+mu2^2
  num = (r2 + C1) * (cD - r2 + C2)
  den = (msq + C1) * (cS - msq + C2)
  result = mean(num/den)

Implementation:
- fp16 conv path: inputs cast fp32->fp16 by gpsimd SWDGE cast-DMA; 11-tap
  separable conv as dense band matmuls (5 overlapping 128-row windows,
  stride 96). Pass 1 convolves H data-stationary (output transposed so W
  lands on partitions); pass 2 convolves W band-stationary with +-
  accumulation pairs.
- Conv streams: x, y, w=x*y (band scaled 2x), s=x^2+y^2 (PSUM-accumulated
  from sx, sy matmuls). Pass-1 emission is stream-major with a per-stream
  PSUM->SBUF drain copy (ACT x3, DVE x1) so the next slot's matmuls only
  wait on the oldest drain.
- Post math: ACT squares (A,B), custom DVE sub (r2), gpsimd add (msq),
  custom DVE (S0-S1+c0)*(S1+c1) for num/den, RECIPROCAL_APPROX_FAST,
  DVE fused multiply+row-sum (accum_out) for the mean.
- fp16 band taps are ulp-nudged so sum(fl16(g)) == 1 (sigma12 is
  first-order sensitive to the weight sum; plain rounding biases the
  mean by several percent).
- Software-pipelined emission (2 window-slot stagger) keeps PE warm.
"""

import math

import numpy as np

from concourse import bacc, bass, mybir, tile
from concourse.bass_utils import run_bass_kernel_spmd

B_FULL, C, H, W = 16, 3, 512, 512
N_CORES = 8
B_LOCAL = B_FULL // N_CORES          # 2
N_PLANES = B_LOCAL * C               # 6 spatial planes per core
KSZ = 11
PAD = KSZ // 2
SSIM_C1 = 0.01 ** 2
SSIM_C2 = 0.03 ** 2

STRIDE = 96
NWIN = 5
CHUNKS = [(0, 101), (101, 96), (197, 96), (293, 96), (389, 123)]
WF = NWIN * 512                      # 2560 free cols per plane

FP32 = mybir.dt.float32
FP16 = mybir.dt.float16

_OPS = {}


def _register_custom_ops():
    """Idempotently register the SSIM num/den custom DVE op."""
    global _OPS
    if _OPS:
        return _OPS
    import concourse.dve_ops as D
    from concourse.dve_spec import Spec, Src0, Src1, C0, C1, lower, _has_src1
    from concourse.dve_uop import DveOpSpec

    def reg(op):
        D.OPS.append(op)
        D._SUB_OPCODE_FOR_NAME[op.name] = D._CUSTOM_DVE_ROW_BASE + len(D.OPS) - 1
        D.CUSTOM_DVE_SPECS[op.name] = op.spec
        for ver in ("v3", "v4"):
            uops = lower(op.spec, ver=ver)
            so = DveOpSpec(name=op.name, opcode=D.get_dve_sub_opcode(op.name),
                           uops=uops, rd1_en=_has_src1(op.spec))
            op.uops_sha[ver] = so.sha(ver)
        return op

    if "SSIM_ND_ANT" in D._SUB_OPCODE_FOR_NAME:
        nd = next(o for o in D.OPS if o.name == "SSIM_ND_ANT")
        sub = next(o for o in D.OPS if o.name == "SSIM_SUB_ANT")
        add = next(o for o in D.OPS if o.name == "SSIM_ADD_ANT")
    else:
        nd = reg(D.DveOp(
            "SSIM_ND_ANT",
            Spec(body=(Src0 - Src1 + C0) * (Src1 + C1),
                 reference=lambda in0, in1, s0, s1, imm2:
                     (in0.astype(np.float32) - in1 + s0)
                     * (in1.astype(np.float32) + s1)),
            subdim=False, uops_sha={}))
        sub = reg(D.DveOp(
            "SSIM_SUB_ANT",
            Spec(body=Src0 - Src1,
                 reference=lambda in0, in1, s0, s1, imm2:
                     in0.astype(np.float32) - in1.astype(np.float32)),
            subdim=False, uops_sha={}))
        add = reg(D.DveOp(
            "SSIM_ADD_ANT",
            Spec(body=Src0 + Src1,
                 reference=lambda in0, in1, s0, s1, imm2:
                     in0.astype(np.float32) + in1.astype(np.float32)),
            subdim=False, uops_sha={}))
    _OPS = {"nd": nd, "sub": sub, "add": add,
            "recip": D.RECIPROCAL_APPROX_FAST,
            "recip_consts": D.RECIP_APPROX_FAST_CONSTS}
    return _OPS


def _gaussian_1d():
    x = np.arange(KSZ)
    g = np.exp(-((x - KSZ // 2) ** 2) / (2.0 * 1.5 ** 2))
    return (g / g.sum()).astype(np.float64)


def _gaussian_1d_f16():
    """fp16 taps nudged by +-1 ulp so sum(fl16(g)) == 1 to ~1e-7.

    sigma12 = conv(xy) - mu1*mu2 is first-order sensitive to the weight sum
    (bias ~ -eps * E[xy]); plain fp16 rounding leaves eps ~ 1e-4 which biases
    the SSIM mean by several percent."""
    g16 = _gaussian_1d().astype(np.float16)
    for _ in range(200):
        e = g16.astype(np.float64).sum() - 1.0
        if abs(e) < 5e-8:
            break
        best = None
        for i in range(KSZ):
            step = np.nextafter(g16[i], np.float16(1.0 if e < 0 else 0.0))
            ne = e + (float(step) - float(g16[i]))
            if best is None or abs(ne) < abs(best[1]):
                best = (i, ne, step)
        i, ne, step = best
        if abs(ne) >= abs(e):
            break
        g16[i] = step
    return g16.astype(np.float64)


def _build_bands():
    """[128, 5*128] f16; window c at cols [128c, 128c+n_c).
    out[s+jj] = sum_r band[r, 128c+jj] * x[96c + r]."""
    g = _gaussian_1d_f16()
    bands = np.zeros((128, NWIN * 128), dtype=np.float64)
    for c, (s, n) in enumerate(CHUNKS):
        r0 = STRIDE * c
        for r in range(128):
            for jj in range(n):
                t = (r0 + r) - (s + jj) + PAD
                if 0 <= t < KSZ:
                    bands[r, c * 128 + jj] = g[t]
    return bands.astype(np.float16)


def _build_graph():
    ops = _register_custom_ops()
    nc = bacc.Bacc()
    img1 = nc.declare_dram_parameter("img1", [B_LOCAL, C, H, W], FP32, isOutput=False)
    img2 = nc.declare_dram_parameter("img2", [B_LOCAL, C, H, W], FP32, isOutput=False)
    bands = nc.declare_dram_parameter("bands", [128, NWIN * 128], FP16, isOutput=False)
    bands2 = nc.declare_dram_parameter("bands2", [128, NWIN * 128], FP16, isOutput=False)
    bandsn = nc.declare_dram_parameter("bandsn", [128, NWIN * 128], FP16, isOutput=False)
    out = nc.declare_dram_parameter("out", [128, N_PLANES * NWIN], FP32, isOutput=True)

    Alu = mybir.AluOpType
    Act = mybir.ActivationFunctionType
    rc = ops["recip_consts"]
    INV_SQRT2 = 1.0 / math.sqrt(2.0)
    QS = {"x": 0, "y": 1, "w": 2, "s": 3}   # stream order in p1/yv blocks

    with tile.TileContext(nc) as tc:
        with (
            tc.tile_pool(name="const_p", bufs=1) as const_p,
            tc.tile_pool(name="in_p", bufs=1) as in_p,
            tc.tile_pool(name="pre_p", bufs=2) as pre_p,
            tc.tile_pool(name="yv_p", bufs=2) as yv_p,
            tc.tile_pool(name="post_p", bufs=2) as post_p,
            tc.tile_pool(name="ps1_p", bufs=1, space="PSUM") as ps1_p,
            tc.tile_pool(name="ps2_p", bufs=1, space="PSUM") as ps2_p,
        ):
            band_t = const_p.tile([128, NWIN * 128], FP16, name="band_t")
            band2_t = const_p.tile([128, NWIN * 128], FP16, name="band2_t")
            bandn_t = const_p.tile([128, NWIN * 128], FP16, name="bandn_t")
            nc.sync.dma_start(out=band_t[:], in_=bands[:, :])
            nc.sync.dma_start(out=band2_t[:], in_=bands2[:, :])
            nc.sync.dma_start(out=bandn_t[:], in_=bandsn[:, :])

            acc = const_p.tile([128, N_PLANES * NWIN], FP32, name="acc")
            nc.vector.memset(acc[:], 0.0)

            # --- input cast-DMAs (fp32 HBM -> fp16 SBUF), one per plane,
            # issued lazily ~one plane ahead from the pipeline loop ---
            x16 = in_p.tile([128, N_PLANES * WF], FP16, name="x16")
            y16 = in_p.tile([128, N_PLANES * WF], FP16, name="y16")

            def emit_load(p):
                for dst, src in ((x16, img1), (y16, img2)):
                    ap = bass.AP(src, p * H * W,
                                 [[W, 128], [STRIDE * W, NWIN], [1, W]])
                    nc.gpsimd.dma_start(
                        out=dst[:, p * WF:(p + 1) * WF].rearrange(
                            "p (c w) -> p c w", c=NWIN),
                        in_=ap)

            emit_load(0)
            emit_load(1)

            pre = {}     # plane -> dict of w16/sx/sy tiles
            yv = {}      # plane -> [128, 5*2048] f16 (per cw: x|y|w|s 512-blocks)

            def emit_pre(p):
                xp = x16[:, p * WF:(p + 1) * WF]
                yp = y16[:, p * WF:(p + 1) * WF]
                w16 = pre_p.tile([128, WF], FP16, name="w16")
                sx = pre_p.tile([128, WF], FP16, name="sx")
                sy = pre_p.tile([128, WF], FP16, name="sy")
                nc.gpsimd.tensor_tensor(w16[:], xp, yp, Alu.mult)
                nc.scalar.activation(sx[:], xp, Act.Square)
                nc.scalar.activation(sy[:], yp, Act.Square)
                pre[p] = {"w": w16, "sx": sx, "sy": sy}

            def emit_pass1(p, cw):
                if cw == 0:
                    yv[p] = yv_p.tile([128, NWIN * 2048], FP16, name="yv")
                pr = pre[p]
                # stream-major: each stream's 5 matmuls then its drain copy,
                # so the next slot's matmuls only wait on the oldest copy.
                p1 = {q: ps1_p.tile([128, 512], FP32, name=f"p1{q}")
                      for q in ("w", "s", "x", "y")}

                def mms(q, src, bandsrc, accum_src=None):
                    for c, (s, n) in enumerate(CHUNKS):
                        col = c * 512 + STRIDE * cw
                        bnd = bandsrc[:, c * 128:c * 128 + n]
                        if accum_src is None:
                            nc.tensor.matmul(
                                p1[q][:, s:s + n], src[:, col:col + 128],
                                bnd, start=True, stop=True)
                        else:
                            nc.tensor.matmul(
                                p1[q][:, s:s + n], src[:, col:col + 128],
                                bnd, start=True, stop=False)
                            nc.tensor.matmul(
                                p1[q][:, s:s + n], accum_src[:, col:col + 128],
                                bnd, start=False, stop=True)

                def drain(q, eng):
                    dst = yv[p][:, cw * 2048 + QS[q] * 512:
                                cw * 2048 + QS[q] * 512 + 512]
                    if eng == "act":
                        nc.scalar.copy(dst, p1[q][:, :])
                    else:
                        nc.vector.tensor_copy(dst, p1[q][:, :])

                # x,y first: they depend only on the input DMA, not the
                # pre-ops, so plane-boundary slots start without waiting on
                # the ACT/gpsimd pre backlog.
                mms("x", x16[:, p * WF:(p + 1) * WF], band_t)
                drain("x", "act")
                mms("y", y16[:, p * WF:(p + 1) * WF], band_t)
                drain("y", "act")
                mms("s", pr["sx"], band_t, accum_src=pr["sy"])
                drain("s", "act")
                mms("w", pr["w"], band2_t)
                drain("w", "dve")

            def emit_pass2_post(p, c2, tail=False):
                s2, n2 = CHUNKS[c2]
                bnd = band_t[:, c2 * 128:c2 * 128 + n2]
                bndn = bandn_t[:, c2 * 128:c2 * 128 + n2]
                ma = ps2_p.tile([128, 512], FP32, name="ma")
                mb = ps2_p.tile([128, 512], FP32, name="mb")
                cd = ps2_p.tile([128, 512], FP32, name="cd")
                cs = ps2_p.tile([128, 512], FP32, name="cs")
                yvx = yv[p][:, c2 * 2048 + QS["x"] * 512:c2 * 2048 + QS["x"] * 512 + 512]
                yvy = yv[p][:, c2 * 2048 + QS["y"] * 512:c2 * 2048 + QS["y"] * 512 + 512]
                yvw = yv[p][:, c2 * 2048 + QS["w"] * 512:c2 * 2048 + QS["w"] * 512 + 512]
                yvs = yv[p][:, c2 * 2048 + QS["s"] * 512:c2 * 2048 + QS["s"] * 512 + 512]
                nc.tensor.matmul(ma[:n2, :], bnd, yvx, start=True, stop=False)
                nc.tensor.matmul(ma[:n2, :], bnd, yvy, start=False, stop=True)
                nc.tensor.matmul(mb[:n2, :], bnd, yvx, start=True, stop=False)
                nc.tensor.matmul(mb[:n2, :], bndn, yvy, start=False, stop=True)
                nc.tensor.matmul(cd[:n2, :], bnd, yvw, start=True, stop=True)
                nc.tensor.matmul(cs[:n2, :], bnd, yvs, start=True, stop=True)

                At = post_p.tile([128, 512], FP16, name="At")
                Bt = post_p.tile([128, 512], FP16, name="Bt")
                r2t = post_p.tile([128, 512], FP16, name="r2t")
                msqt = post_p.tile([128, 512], FP16, name="msqt")
                numt = post_p.tile([128, 512], FP16, name="numt")
                dent = post_p.tile([128, 512], FP32, name="dent")
                rect = post_p.tile([128, 512], FP16, name="rect")
                scr = post_p.tile([128, 512], FP16, name="scr")
                nc.scalar.activation(At[:n2, :], ma[:n2, :], Act.Square,
                                     scale=INV_SQRT2)
                nc.scalar.activation(Bt[:n2, :], mb[:n2, :], Act.Square,
                                     scale=INV_SQRT2)
                nc.vector._custom_dve(
                    ops["sub"], out=r2t[:n2, :], in0=At[:n2, :], in1=Bt[:n2, :])
                if tail:
                    # gpsimd TT is ~1.9us and serializes the drain of the
                    # final staggered chunks; DVE custom add is ~0.7us.
                    nc.vector._custom_dve(
                        ops["add"], out=msqt[:n2, :], in0=At[:n2, :],
                        in1=Bt[:n2, :])
                else:
                    nc.gpsimd.tensor_tensor(
                        msqt[:n2, :], At[:n2, :], Bt[:n2, :], Alu.add)
                nc.vector._custom_dve(
                    ops["nd"], out=numt[:n2, :], in0=cd[:n2, :], in1=r2t[:n2, :],
                    s0=SSIM_C2, s1=SSIM_C1)
                nc.vector._custom_dve(
                    ops["nd"], out=dent[:n2, :], in0=cs[:n2, :], in1=msqt[:n2, :],
                    s0=SSIM_C2, s1=SSIM_C1)
                nc.vector._custom_dve(
                    ops["recip"], out=rect[:n2, :], in0=dent[:n2, :],
                    s0=rc["s0"], s1=rc["s1"], imm2=rc["imm2"])
                k = p * NWIN + c2
                nc.vector.scalar_tensor_tensor(
                    scr[:n2, :], numt[:n2, :], 0.0, rect[:n2, :],
                    Alu.add, Alu.mult, accum_out=acc[:n2, k:k + 1])

            from collections import deque
            pending = deque()
            for p in range(N_PLANES):
                for cw in range(NWIN):
                    if p == 0 and cw == 0:
                        emit_pre(0)
                    emit_pass1(p, cw)
                    pending.append((p, cw))
                    if len(pending) > 2:
                        emit_pass2_post(*pending.popleft())
                    if cw == 1 and p + 2 < N_PLANES:
                        emit_load(p + 2)
                    if cw == 1 and p + 1 < N_PLANES:
                        emit_pre(p + 1)
            while pending:
                emit_pass2_post(*pending.popleft(), tail=True)

            nc.sync.dma_start(out=out[:, :], in_=acc[:, :])

    nc.compile()
    return nc


_NC_CACHE = None


def _in_maps(img1, img2):
    img1 = np.ascontiguousarray(img1, dtype=np.float32)
    img2 = np.ascontiguousarray(img2, dtype=np.float32)
    bands = _build_bands()
    b32 = bands.astype(np.float32)
    return [
        {
            "img1": img1[i * B_LOCAL:(i + 1) * B_LOCAL],
            "img2": img2[i * B_LOCAL:(i + 1) * B_LOCAL],
            "bands": bands,
            "bands2": (b32 * 2.0).astype(np.float16),
            "bandsn": (-b32).astype(np.float16),
        }
        for i in range(N_CORES)
    ]


def kernel(img1: np.ndarray, img2: np.ndarray) -> np.ndarray:
    global _NC_CACHE
    if _NC_CACHE is None:
        _NC_CACHE = _build_graph()
    nc = _NC_CACHE

    res = run_bass_kernel_spmd(nc, _in_maps(img1, img2), list(range(N_CORES)))
    total = np.float64(0.0)
    for r in res.results:
        total += np.asarray(r["out"], dtype=np.float64).sum()
    mean = total / (B_FULL * C * H * W)
    return np.array(mean, dtype=np.float32)

